# revision 39
# baseline (speedup 1.0000x reference)
"""Trainium2 Bass kernel for nn_BodyKinematics (batched tree forward kinematics).

Contract: kernel(**inputs) takes the FULL unsharded inputs as numpy arrays and
returns the FULL output (B, N, 4, 4) float32.  Batch is sharded across 8
NeuronCores (pure data parallelism); per-edge parameters replicated.

Math (matches the jax reference):
  theta = tanh(log_angles) * scale                     # offset == 0 fast path
  per edge e: r = Rx @ Ry @ Rz ; local = r @ tip       # affine 3x4
  tree: W_n = W_parent(n) @ local_{n-1}, parent(n) = (n-1)//2

Device layout (fp16; per core 512 batch rows = NSUB subtiles x 128 partitions
x S_B "lanes"; partition = batch-within-subtile, lanes = extra batch rows
interleaved into the free dim so tree ops fuse across them):
  e' = S_B*e + lane   (interleaved edge index, M = S_B*E per subtile)
  theta/trig tiles:  [128, 3*M]       axis-major, e' innermost (packed)
  locT tile:         [128, 12*M]      k*4M + l*M + e'   (BC output, packed)
  lR tile:           [128, 36*M]      e'*36 + k*12 + l*3 + i  (locals
                      replicated x3 over i; needed so lane-fused tree muls
                      fit the 3-free-dim AP limit)
  w tile:            [128, 255*12*S_B] node n>=1 at (n-1)*12S_B + lane*12
                      + l*3 + i      (TRANSPOSED 3x4: (l,i), no bottom row)
NOTE (measured on HW): DVE runs ~1 elem/cycle/partition regardless of dtype
(the cost model's fp16 2x_1p mode does NOT engage); fp16 still halves DMA
traffic and SBUF footprint.  GPSIMD costs ~1.14us/op + 0.53ns/elem with a
cliff above ~1024-elem ops, so it gets few, chunky op slices.
Output DMA'd as fp16 in device layout; host unpacks to (B,N,4,4) fp32.
"""

import os
import sys

for _p in ("/opt/trn_rl_repo",):
    if _p not in sys.path and os.path.isdir(_p):
        sys.path.insert(0, _p)

import numpy as np

B, E, N = 4096, 255, 256
J = 3 * E
NCORE, P = 8, 128
S_B = 2                 # batch lanes interleaved per subtile
NSUB = 4 // S_B         # subtiles per core
M = S_B * E             # interleaved edges per subtile
BPC = P * S_B * NSUB    # 512 batch rows per core
OUTC = NSUB * 255 * 12 * S_B   # out cols per partition row (fp16)
PI = float(np.pi)

# ---- engine split knobs ----
# POOL economics (measured): ~1.14us fixed + ~0.53ns/elem, cliff above
# ~1024-elem ops.  Give POOL few ops of ~800-1536 elems.
BC_POOL_TRIPLES = {2: 0.30, 3: 0.30, 4: 0.40, 5: 0.40}  # triple idx -> e'-col frac on POOL
TREE_POOL_FRAC = 0.10   # node-range tail of tree muls -> POOL (off)
TREE_POOL_MIN_M = 16
TREE_ADD_POOL_MIN_M = 9999  # levels this big run their k-adds on POOL
REP_SPLIT = (1.0, 0.0, 0.0)   # lR replication: ACT, POOL, DVE shares
LAST_LEVEL_POOL = False
TR_POOL = False  # tr side-chain on POOL regressed (small-op + sync cost)
F16 = True              # compute dtype on device (False -> fp32 A/B test)

_state: dict = {}


# --------------------------------------------------------------------------- #
# numpy fallback (exact float32 port of the reference)
# --------------------------------------------------------------------------- #
def _np_skew(a):
    x, y, z = a[..., 0], a[..., 1], a[..., 2]
    zero = np.zeros_like(x)
    return np.stack([
        np.stack([zero, -z, y], -1),
        np.stack([z, zero, -x], -1),
        np.stack([-y, x, zero], -1)], -2)


def _np_fallback(log_angles, tip_to_base, rot_axes, rot_constraints):
    la = log_angles.astype(np.float32)
    b, e3 = la.shape
    e = e3 // 3
    n = e + 1
    theta = np.tanh(la) * rot_constraints[:, 0] + rot_constraints[:, 1]
    K = _np_skew(rot_axes.astype(np.float32))
    K2 = np.einsum('mij,mjk->mik', K, K).astype(np.float32)
    s = np.sin(theta)[..., None, None]
    c = (1.0 - np.cos(theta))[..., None, None]
    I3 = np.eye(3, dtype=np.float32)
    rots = (I3 + s * K + c * K2).reshape(b, e, 3, 3, 3).astype(np.float32)
    r = np.einsum('beij,bejk,bekl->beil', rots[:, :, 0], rots[:, :, 1],
                  rots[:, :, 2]).astype(np.float32)
    T = np.zeros((b, e, 4, 4), np.float32)
    T[..., :3, :3] = r
    T[..., 3, 3] = 1.0
    local = np.einsum('beij,ejk->beik', T,
                      tip_to_base.astype(np.float32)).astype(np.float32)
    worlds = np.zeros((b, n, 4, 4), np.float32)
    worlds[:, 0] = np.eye(4, dtype=np.float32)
    for i in range(1, n):
        par = (i - 1) // 2
        worlds[:, i] = (worlds[:, par] @ local[:, i - 1]).astype(np.float32)
    return worlds


# --------------------------------------------------------------------------- #
# device kernel build
# --------------------------------------------------------------------------- #
def _build_nc(sc_const: float, loop_n: int = 1):
    import concourse.bacc as bacc
    import concourse.mybir as mybir
    from concourse.tile import TileContext
    import concourse.bass as bass
    from contextlib import ExitStack

    f32 = mybir.dt.float32
    f16 = mybir.dt.float16 if F16 else mybir.dt.float32
    i16 = mybir.dt.int16 if F16 else mybir.dt.int32
    MASK = 0x7FFF if F16 else 0x7FFFFFFF
    Alu = mybir.AluOpType
    AFT = mybir.ActivationFunctionType

    nc = bacc.Bacc("TRN2", target_bir_lowering=False, debug=False)

    la_d = nc.dram_tensor("la", [BPC, J], f32, kind="ExternalInput")
    tip_d = nc.dram_tensor("tipT", [1, 12 * M], f16, kind="ExternalInput")  # compute dtype
    out_d = nc.dram_tensor("out", [P, OUTC], f16, kind="ExternalOutput")

    def AP(t, off, dims):
        return bass.AP(t, off, dims)

    SB12 = 12 * S_B

    with TileContext(nc) as tc:
        with tc.tile_pool(name="main", bufs=1) as pool, \
             ExitStack() as _loop_ctx:
            if loop_n > 1:
                _loop_ctx.enter_context(tc.For_i(0, loop_n, 1))

            la_t = [pool.tile([P, S_B * J], f32, name=f"la{s}")
                    for s in range(NSUB)]
            th_t = [pool.tile([P, 3 * M], f16, name=f"th{s}")
                    for s in range(NSUB)]
            ab_t = [pool.tile([P, 3 * M], f16, name=f"ab{s}")
                    for s in range(NSUB)]
            sin_t = [pool.tile([P, 3 * M], f16, name=f"sin{s}")
                     for s in range(NSUB)]
            cos_t = [pool.tile([P, 3 * M], f16, name=f"cos{s}")
                     for s in range(NSUB)]
            tip_t = pool.tile([P, 12 * M], f16, name="tipT")
            loc_t = [pool.tile([P, 12 * M], f16, name=f"locT{s}")
                     for s in range(NSUB)]
            lR_t = [pool.tile([P, 36 * M], f16, name=f"lR{s}")
                    for s in range(NSUB)]
            w_t = [pool.tile([P, 255 * SB12], f16, name=f"w{s}")
                   for s in range(NSUB)]
            t_t = [pool.tile([P, 2 * 64 * SB12], f16, name=f"t{s}")
                   for s in range(NSUB)]
            # BC scratch shared across subs (lifetimes serialize)
            tA_t = pool.tile([P, 4 * M], f16, tag="tA", name="tA")
            r0_t = pool.tile([P, 4 * M], f16, tag="r0", name="r0")
            r1_t = pool.tile([P, 4 * M], f16, tag="r1", name="r1")
            q2_t = pool.tile([P, 4 * M], f16, tag="q2", name="q2")
            hpi_t = pool.tile([P, 1], f32, name="hpi")

            nc.gpsimd.memset(hpi_t[:], PI / 2.0)
            # warm the ACT function tables while input DMAs run
            warm_t = pool.tile([P, 1], f32, name="warm")
            nc.scalar.activation(warm_t[:], hpi_t[:], AFT.Tanh)
            nc.scalar.activation(warm_t[:], hpi_t[:], AFT.Sin)

            # ---------------- input DMAs ----------------
            la_v = la_d[:].rearrange("(s l p) j -> p s l j", p=P, l=S_B)
            for s in range(NSUB):
                for ln in range(S_B):
                    nc.sync.dma_start(la_t[s][:, ln * J:(ln + 1) * J],
                                      la_v[:, s, ln])
            tq = (12 * M) // 4
            for c in range(4):
                tip_src = AP(tip_d, c * tq, [[0, P], [1, tq]])
                nc.sync.dma_start(tip_t[:, c * tq:(c + 1) * tq], tip_src)

            act = nc.scalar.activation

            for s in range(NSUB):
                lat = la_t[s][:]
                tht = th_t[s][:]
                # tanh with axis-deinterleave: iterate (a, e, lane)
                th_out = AP(tht.tensor, tht.offset,
                            [list(tht.ap[0]), [M, 3], [S_B, E], [1, S_B]])
                la_in = AP(lat.tensor, lat.offset,
                           [list(lat.ap[0]), [1, 3], [3, E], [J, S_B]])
                act(th_out, la_in, AFT.Tanh)
                # per-axis |theta| + sin/cos, all on ACT (abs/sin/tanh
                # share one ACT table set); z first (BC triples 1-2 need
                # only the z trig)
                for a in (2, 1, 0):
                    sl = slice(a * M, (a + 1) * M)
                    act(ab_t[s][:, sl], th_t[s][:, sl], AFT.Abs)
                    act(sin_t[s][:, sl], th_t[s][:, sl], AFT.Sin,
                        scale=sc_const)
                    act(cos_t[s][:, sl], ab_t[s][:, sl], AFT.Sin,
                        bias=hpi_t[:], scale=-sc_const)

            # ---------------- BC: locals into locT layout ----------------
            def trig4(tile, axis):
                t = tile[:]
                return AP(t.tensor, t.offset + axis * M,
                          [list(t.ap[0]), [0, 4], [1, M]])

            def row4(tile, k):
                t = tile[:]
                return AP(t.tensor, t.offset + k * 4 * M,
                          [list(t.ap[0]), [M, 4], [1, M]])

            def scr4(tile):
                t = tile[:]
                return AP(t.tensor, t.offset,
                          [list(t.ap[0]), [M, 4], [1, M]])

            def col_slice(ap, lo_e, n_e):
                dims = [list(d) for d in ap.ap]
                dims[-1] = [dims[-1][0], n_e]
                return AP(ap.tensor, ap.offset + lo_e, dims)

            for s in range(NSUB):
                sx, sy, sz = (trig4(sin_t[s], a) for a in range(3))
                cx, cy, cz = (trig4(cos_t[s], a) for a in range(3))
                T0, T1, T2 = (row4(tip_t, k) for k in range(3))
                L0, L1, L2 = (row4(loc_t[s], k) for k in range(3))
                r0v, r1v, q2v, tAv = (scr4(t) for t in
                                      (r0_t, r1_t, q2_t, tA_t))

                triples = [
                    (cz, T0, sz, T1, r0v, Alu.subtract, True),
                    (sz, T0, cz, T1, r1v, Alu.add, False),
                    (cy, r0v, sy, T2, L0, Alu.add, False),
                    (sy, r0v, cy, T2, q2v, Alu.subtract, False),
                    (cx, r1v, sx, q2v, L1, Alu.subtract, True),
                    (sx, r1v, cx, q2v, L2, Alu.add, False),
                ]
                for ti, (a, b, c, d, dst, op, ta_first) in enumerate(triples):
                    pf = BC_POOL_TRIPLES.get(ti, 0.0)
                    m_dve = M - int(M * pf)
                    for eng, lo_e, n_e in ((nc.vector, 0, m_dve),
                                           (nc.gpsimd, m_dve, M - m_dve)):
                        if n_e <= 0:
                            continue
                        tt = eng.tensor_tensor
                        aa, bb, cc, dd, dd_dst, tv = (
                            col_slice(x, lo_e, n_e)
                            for x in (a, b, c, d, dst, tAv))
                        tt(tv, aa, bb, Alu.mult)
                        tt(dd_dst, cc, dd, Alu.mult)
                        if ta_first:
                            tt(dd_dst, tv, dd_dst, op)
                        else:
                            tt(dd_dst, dd_dst, tv, op)

            # ---------------- lR replication ----------------
            rep_engs = []
            acc = 0.0
            for frac, eng in zip(REP_SPLIT, ("act", "pool", "dve")):
                lo_e = int(M * acc)
                acc += frac
                hi_e = M if acc >= 0.999 else int(M * acc)
                if hi_e > lo_e:
                    rep_engs.append((eng, lo_e, hi_e))

            # split each rep op at edge 127 (e' = 127*S_B): the FRONT range
            # covers everything tree levels 1-5 read, so the tree can start
            # as soon as the front lands; the BACK replicates during the
            # tree's early levels.  Removes the DVE stall where rep0 (one
            # big op per k) outlasted the BC work available to hide it.
            REP_CUT = 127 * S_B
            for s in range(NSUB):
                lt = loc_t[s][:]
                rt = lR_t[s][:]
                for k in range(3):
                    for eng, lo_e, hi_e in rep_engs:
                        cuts = [lo_e]
                        if lo_e < REP_CUT < hi_e:
                            cuts.append(REP_CUT)
                        cuts.append(hi_e)
                        for c0, c1 in zip(cuts[:-1], cuts[1:]):
                            n_e = c1 - c0
                            dst = AP(rt.tensor,
                                     rt.offset + c0 * 36 + k * 12,
                                     [list(rt.ap[0]), [36, n_e], [3, 4],
                                      [1, 3]])
                            src = AP(lt.tensor,
                                     lt.offset + k * 4 * M + c0,
                                     [list(lt.ap[0]), [1, n_e], [M, 4],
                                      [0, 3]])
                            if eng == "act":
                                nc.scalar.copy(dst, src)
                            elif eng == "pool":
                                nc.gpsimd.tensor_copy(dst, src)
                            else:
                                nc.vector.tensor_copy(dst, src)

            # ---------------- tree ----------------
            for s in range(NSUB):
                wt = w_t[s][:]
                rt = lR_t[s][:]
                tst = t_t[s][:]
                wpd, rpd, tpd = (list(x.ap[0]) for x in (wt, rt, tst))

                def wAP(off, dims):
                    return AP(wt.tensor, wt.offset + off, [list(wpd)] + dims)

                def rAP(off, dims):
                    return AP(rt.tensor, rt.offset + off, [list(rpd)] + dims)

                def tAP(off, dims):
                    return AP(tst.tensor, tst.offset + off,
                              [list(tpd)] + dims)

                def nd(n):
                    return (n - 1) * SB12

                # nodes 1,2 <- locals of edges 0,1 ((l,i) from lR's (k,l))
                for n in (1, 2):
                    dst = wAP(nd(n), [[12, S_B], [3, 4], [1, 3]])
                    src = rAP((n - 1) * S_B * 36,
                              [[36, S_B], [3, 4], [12, 3]])
                    nc.vector.tensor_copy(dst, src)

                levels = [(3, 7), (7, 15), (15, 31), (31, 63), (63, 127),
                          (127, 191), (191, 255), (255, 256)]

                def emit_group(eng, glo, ghi, lo, add_eng=None,
                               tr_eng=None):
                    """Full level-compute for child-node range [glo, ghi).

                    k=1/k=2 partials go to k-separated halves of the t
                    scratch so the k=2 muls (eng) never overlap the k=1
                    add (add_eng) — no cross-engine WAR stall.
                    """
                    gm = ghi - glo
                    if gm <= 0:
                        return
                    tt = eng.tensor_tensor
                    tta = (add_eng or eng).tensor_tensor
                    for k in range(3):
                        for side in (0, 1):
                            q = (gm + (1 - side)) // 2
                            if q <= 0:
                                continue
                            gplo = (glo + side - 1) // 2
                            in0 = wAP(nd(gplo) + k * 3,
                                      [[12, S_B * q], [0, 4], [1, 3]])
                            e0 = glo + side - 1
                            in1 = rAP(S_B * e0 * 36 + k * 12,
                                      [[72 * S_B, q], [36, S_B], [1, 12]])
                            if k == 0:
                                dst = wAP(nd(glo + side),
                                          [[24 * S_B, q], [12, S_B],
                                           [1, 12]])
                            else:
                                dst = tAP((k - 1) * 64 * SB12
                                          + (glo - lo + side) * SB12,
                                          [[24 * S_B, q], [12, S_B],
                                           [1, 12]])
                            tt(dst, in0, in1, Alu.mult)
                        if k > 0:
                            wa = wAP(nd(glo), [[1, SB12 * gm]])
                            ta = tAP((k - 1) * 64 * SB12
                                     + (glo - lo) * SB12, [[1, SB12 * gm]])
                            tta(wa, wa, ta, Alu.add)
                    # translation add: w[child].t += w[parent].t
                    # (a trailing side-chain: next level's muls read only
                    # rotation rows, so tr can run on another engine)
                    ttr = (tr_eng or eng).tensor_tensor
                    for side in (0, 1):
                        q = (gm + (1 - side)) // 2
                        if q <= 0:
                            continue
                        gplo = (glo + side - 1) // 2
                        wtr = wAP(nd(glo + side) + 9,
                                  [[24 * S_B, q], [12, S_B], [1, 3]])
                        ptr = wAP(nd(gplo) + 9, [[12, S_B * q], [1, 3]])
                        ttr(wtr, wtr, ptr, Alu.add)

                for (lo, hi) in levels:
                    m = hi - lo
                    if LAST_LEVEL_POOL and lo >= 191:
                        # whole tail level on POOL: frees DVE at the
                        # iteration tail; latency hides under the next
                        # iteration's (DVE-only) BC phase.
                        emit_group(nc.gpsimd, lo, hi, lo)
                        continue
                    if m >= TREE_POOL_MIN_M and TREE_POOL_FRAC > 0:
                        mid = hi - int(m * TREE_POOL_FRAC)
                        mid += (hi - mid) % 2
                    else:
                        mid = hi
                    a_eng = (nc.gpsimd if m >= TREE_ADD_POOL_MIN_M
                             else None)
                    t_eng = nc.gpsimd if TR_POOL else None
                    emit_group(nc.vector, lo, mid, lo, add_eng=a_eng,
                               tr_eng=t_eng)
                    emit_group(nc.gpsimd, mid, hi, lo)

            # ---------------- output DMAs ----------------
            # staged: nodes 1..126 (cols [0, 126*SB12)) are final after level
            # (63,127); tile deps let those DMAs start while the tree tail
            # still runs.  Remainder flushed after the full tree.
            ov = out_d[:]
            w_cols = 255 * SB12
            c1 = 126 * SB12    # nodes 1..126: final after level (63,127)
            c2 = 190 * SB12    # nodes 127..190: final after (127,191)
            for s in range(NSUB):
                for c0, cn in ((0, c1), (c1, c2 - c1), (c2, w_cols - c2)):
                    dst = AP(ov.tensor, ov.offset + s * w_cols + c0,
                             [list(ov.ap[0]), [1, cn]])
                    nc.sync.dma_start(dst, w_t[s][:, c0:c0 + cn])

    nc.compile()
    return nc


# --------------------------------------------------------------------------- #
# cached PJRT runner (axon path) — compile once, execute per call
# --------------------------------------------------------------------------- #
def _get_runner(sc_const, loop_n=1):
    key = ("runner", round(sc_const, 6), loop_n)
    if key in _state:
        return _state[key]

    import jax
    from jax.sharding import Mesh, PartitionSpec, NamedSharding
    from jax.experimental.shard_map import shard_map
    import concourse.mybir as mybir
    from concourse import bass2jax

    nc = _build_nc(sc_const, loop_n)
    bass2jax.install_neuronx_cc_hook()

    part_name = (nc.partition_id_tensor.name
                 if nc.partition_id_tensor is not None else None)
    in_names, out_names, out_avals = [], [], []
    for alloc in nc.m.functions[0].allocations:
        if not isinstance(alloc, mybir.MemoryLocationSet):
            continue
        name = alloc.memorylocations[0].name
        if alloc.kind == "ExternalInput":
            if name != part_name:
                in_names.append(name)
        elif alloc.kind == "ExternalOutput":
            out_names.append(name)
            out_avals.append(jax.core.ShapedArray(
                tuple(alloc.tensor_shape), mybir.dt.np(alloc.dtype)))
    n_params = len(in_names)
    all_in = in_names + out_names
    if part_name is not None:
        all_in = all_in + [part_name]

    def _body(*args):
        operands = list(args)
        if part_name is not None:
            operands.append(bass2jax.partition_id_tensor())
        outs = bass2jax._bass_exec_p.bind(
            *operands,
            out_avals=tuple(out_avals),
            in_names=tuple(all_in),
            out_names=tuple(out_names),
            lowering_input_output_aliases=(),
            sim_require_finite=True,
            sim_require_nnan=True,
            nc=nc,
        )
        return tuple(outs)

    devices = jax.devices()[:NCORE]
    mesh = Mesh(np.asarray(devices), ("core",))
    nin = n_params + len(out_names)
    sharded = jax.jit(
        shard_map(_body, mesh=mesh,
                  in_specs=(PartitionSpec("core"),) * nin,
                  out_specs=(PartitionSpec("core"),) * len(out_names),
                  check_rep=False),
        donate_argnums=tuple(range(n_params, nin)),
        keep_unused=True,
    )
    shard0 = NamedSharding(mesh, PartitionSpec("core"))

    def _make_zeros():
        return jax.jit(
            lambda: jax.numpy.zeros((NCORE * P, OUTC),
                        np.float16 if F16 else np.float32),
            out_shardings=shard0)()

    runner = (sharded, in_names, _make_zeros)
    _state[key] = runner
    return runner


def _prep_tip(tip_to_base):
    # tipT[k, l, e'] with e' = S_B*e + lane (lane-duplicated), compute dtype
    cdt = np.float16 if F16 else np.float32
    tip_rows = tip_to_base[:, :3, :].astype(cdt)             # (E, 3, 4)
    tipT = np.repeat(tip_rows.transpose(1, 2, 0), S_B, axis=-1)  # (3,4,M)
    return np.ascontiguousarray(tipT.reshape(1, 12 * M))


def _run_device(log_angles, tip_to_base, sc_const):
    sharded, in_names, make_zeros = _get_runner(sc_const)
    feed = {
        "la": np.ascontiguousarray(log_angles, dtype=np.float32),
        "tipT": np.broadcast_to(_prep_tip(tip_to_base),
                                (NCORE, 12 * M)).copy(),
    }
    args = [feed[name] for name in in_names]
    out = np.asarray(sharded(*args, make_zeros())[0])
    # out: (NCORE*P, OUTC) fp16, device layout -> (B, N, 4, 4) fp32
    v = out.reshape(NCORE, P, NSUB, 255, S_B, 4, 3)  # (c, p, s, n, lane, l, i)
    rot = v.transpose(0, 2, 4, 1, 3, 6, 5)           # (c, s, lane, p, n, i, l)
    res = np.zeros((B, N, 4, 4), np.float32)
    res[:, 1:, :3, :] = rot.reshape(B, 255, 3, 4).astype(np.float32)
    res[:, 0, 0, 0] = 1.0
    res[:, 0, 1, 1] = 1.0
    res[:, 0, 2, 2] = 1.0
    res[:, :, 3, 3] = 1.0
    return res


# --------------------------------------------------------------------------- #
# public entry point
# --------------------------------------------------------------------------- #
def kernel(log_angles, tip_to_base, rot_axes, rot_constraints):
    log_angles = np.asarray(log_angles)
    tip_to_base = np.asarray(tip_to_base)
    rot_axes = np.asarray(rot_axes)
    rot_constraints = np.asarray(rot_constraints)

    expected_shapes = (log_angles.shape == (B, J)
                       and tip_to_base.shape == (E, 4, 4)
                       and rot_axes.shape == (J, 3)
                       and rot_constraints.shape == (J, 2))
    eye_tiled = np.tile(np.eye(3, dtype=np.float32), (E, 1)) \
        if expected_shapes else None
    euler = expected_shapes and np.allclose(rot_axes, eye_tiled, atol=1e-6)
    if not euler:
        return _np_fallback(log_angles, tip_to_base, rot_axes, rot_constraints)

    sc = rot_constraints[:, 0].astype(np.float32)
    of = rot_constraints[:, 1].astype(np.float32)
    const_ok = (np.all(sc == sc[0]) and np.all(of == 0.0)
                and float(sc[0]) > 1e-3
                and abs(float(sc[0])) <= PI + 1e-4)
    if not const_ok:
        return _np_fallback(log_angles, tip_to_base, rot_axes,
                            rot_constraints)

    return _run_device(log_angles, tip_to_base, float(sc[0]))


# revision 40
# speedup vs baseline: 1.0022x; 1.0022x over previous
"""Trainium2 Bass kernel for nn_BodyKinematics (batched tree forward kinematics).

Contract: kernel(**inputs) takes the FULL unsharded inputs as numpy arrays and
returns the FULL output (B, N, 4, 4) float32.  Batch is sharded across 8
NeuronCores (pure data parallelism); per-edge parameters replicated.

Math (matches the jax reference):
  theta = tanh(log_angles) * scale                     # offset == 0 fast path
  per edge e: r = Rx @ Ry @ Rz ; local = r @ tip       # affine 3x4
  tree: W_n = W_parent(n) @ local_{n-1}, parent(n) = (n-1)//2

Device layout (fp16; per core 512 batch rows = NSUB subtiles x 128 partitions
x S_B "lanes"; partition = batch-within-subtile, lanes = extra batch rows
interleaved into the free dim so tree ops fuse across them):
  e' = S_B*e + lane   (interleaved edge index, M = S_B*E per subtile)
  theta/trig tiles:  [128, 3*M]       axis-major, e' innermost (packed)
  locT tile:         [128, 12*M]      k*4M + l*M + e'   (BC output, packed)
  lR tile:           [128, 36*M]      e'*36 + k*12 + l*3 + i  (locals
                      replicated x3 over i; needed so lane-fused tree muls
                      fit the 3-free-dim AP limit)
  w tile:            [128, 255*12*S_B] node n>=1 at (n-1)*12S_B + lane*12
                      + l*3 + i      (TRANSPOSED 3x4: (l,i), no bottom row)
NOTE (measured on HW): DVE runs ~1 elem/cycle/partition regardless of dtype
(the cost model's fp16 2x_1p mode does NOT engage); fp16 still halves DMA
traffic and SBUF footprint.  GPSIMD costs ~1.14us/op + 0.53ns/elem with a
cliff above ~1024-elem ops, so it gets few, chunky op slices.
Output DMA'd as fp16 in device layout; host unpacks to (B,N,4,4) fp32.
"""

import os
import sys

for _p in ("/opt/trn_rl_repo",):
    if _p not in sys.path and os.path.isdir(_p):
        sys.path.insert(0, _p)

import numpy as np

B, E, N = 4096, 255, 256
J = 3 * E
NCORE, P = 8, 128
S_B = 2                 # batch lanes interleaved per subtile
NSUB = 4 // S_B         # subtiles per core
M = S_B * E             # interleaved edges per subtile
BPC = P * S_B * NSUB    # 512 batch rows per core
OUTC = NSUB * 255 * 12 * S_B   # out cols per partition row (fp16)
PI = float(np.pi)

# ---- engine split knobs ----
# POOL economics (measured): ~1.14us fixed + ~0.53ns/elem, cliff above
# ~1024-elem ops.  Give POOL few ops of ~800-1536 elems.
BC_POOL_TRIPLES = {2: 0.30, 3: 0.30, 4: 0.40, 5: 0.40}  # triple idx -> e'-col frac on POOL
TREE_POOL_FRAC = 0.14   # node-range tail of tree muls -> POOL (off)
TREE_POOL_MIN_M = 16
TREE_ADD_POOL_MIN_M = 9999  # levels this big run their k-adds on POOL
REP_SPLIT = (1.0, 0.0, 0.0)   # lR replication: ACT, POOL, DVE shares
LAST_LEVEL_POOL = False
TR_POOL = False  # tr side-chain on POOL regressed (small-op + sync cost)
F16 = True              # compute dtype on device (False -> fp32 A/B test)

_state: dict = {}


# --------------------------------------------------------------------------- #
# numpy fallback (exact float32 port of the reference)
# --------------------------------------------------------------------------- #
def _np_skew(a):
    x, y, z = a[..., 0], a[..., 1], a[..., 2]
    zero = np.zeros_like(x)
    return np.stack([
        np.stack([zero, -z, y], -1),
        np.stack([z, zero, -x], -1),
        np.stack([-y, x, zero], -1)], -2)


def _np_fallback(log_angles, tip_to_base, rot_axes, rot_constraints):
    la = log_angles.astype(np.float32)
    b, e3 = la.shape
    e = e3 // 3
    n = e + 1
    theta = np.tanh(la) * rot_constraints[:, 0] + rot_constraints[:, 1]
    K = _np_skew(rot_axes.astype(np.float32))
    K2 = np.einsum('mij,mjk->mik', K, K).astype(np.float32)
    s = np.sin(theta)[..., None, None]
    c = (1.0 - np.cos(theta))[..., None, None]
    I3 = np.eye(3, dtype=np.float32)
    rots = (I3 + s * K + c * K2).reshape(b, e, 3, 3, 3).astype(np.float32)
    r = np.einsum('beij,bejk,bekl->beil', rots[:, :, 0], rots[:, :, 1],
                  rots[:, :, 2]).astype(np.float32)
    T = np.zeros((b, e, 4, 4), np.float32)
    T[..., :3, :3] = r
    T[..., 3, 3] = 1.0
    local = np.einsum('beij,ejk->beik', T,
                      tip_to_base.astype(np.float32)).astype(np.float32)
    worlds = np.zeros((b, n, 4, 4), np.float32)
    worlds[:, 0] = np.eye(4, dtype=np.float32)
    for i in range(1, n):
        par = (i - 1) // 2
        worlds[:, i] = (worlds[:, par] @ local[:, i - 1]).astype(np.float32)
    return worlds


# --------------------------------------------------------------------------- #
# device kernel build
# --------------------------------------------------------------------------- #
def _build_nc(sc_const: float, loop_n: int = 1):
    import concourse.bacc as bacc
    import concourse.mybir as mybir
    from concourse.tile import TileContext
    import concourse.bass as bass
    from contextlib import ExitStack

    f32 = mybir.dt.float32
    f16 = mybir.dt.float16 if F16 else mybir.dt.float32
    i16 = mybir.dt.int16 if F16 else mybir.dt.int32
    MASK = 0x7FFF if F16 else 0x7FFFFFFF
    Alu = mybir.AluOpType
    AFT = mybir.ActivationFunctionType

    nc = bacc.Bacc("TRN2", target_bir_lowering=False, debug=False)

    la_d = nc.dram_tensor("la", [BPC, J], f32, kind="ExternalInput")
    tip_d = nc.dram_tensor("tipT", [1, 12 * M], f16, kind="ExternalInput")  # compute dtype
    out_d = nc.dram_tensor("out", [P, OUTC], f16, kind="ExternalOutput")

    def AP(t, off, dims):
        return bass.AP(t, off, dims)

    SB12 = 12 * S_B

    with TileContext(nc) as tc:
        with tc.tile_pool(name="main", bufs=1) as pool, \
             ExitStack() as _loop_ctx:
            if loop_n > 1:
                _loop_ctx.enter_context(tc.For_i(0, loop_n, 1))

            la_t = [pool.tile([P, S_B * J], f32, name=f"la{s}")
                    for s in range(NSUB)]
            th_t = [pool.tile([P, 3 * M], f16, name=f"th{s}")
                    for s in range(NSUB)]
            ab_t = [pool.tile([P, 3 * M], f16, name=f"ab{s}")
                    for s in range(NSUB)]
            sin_t = [pool.tile([P, 3 * M], f16, name=f"sin{s}")
                     for s in range(NSUB)]
            cos_t = [pool.tile([P, 3 * M], f16, name=f"cos{s}")
                     for s in range(NSUB)]
            tip_t = pool.tile([P, 12 * M], f16, name="tipT")
            loc_t = [pool.tile([P, 12 * M], f16, name=f"locT{s}")
                     for s in range(NSUB)]
            lR_t = [pool.tile([P, 36 * M], f16, name=f"lR{s}")
                    for s in range(NSUB)]
            w_t = [pool.tile([P, 255 * SB12], f16, name=f"w{s}")
                   for s in range(NSUB)]
            t_t = [pool.tile([P, 2 * 64 * SB12], f16, name=f"t{s}")
                   for s in range(NSUB)]
            # BC scratch shared across subs (lifetimes serialize)
            tA_t = pool.tile([P, 4 * M], f16, tag="tA", name="tA")
            r0_t = pool.tile([P, 4 * M], f16, tag="r0", name="r0")
            r1_t = pool.tile([P, 4 * M], f16, tag="r1", name="r1")
            q2_t = pool.tile([P, 4 * M], f16, tag="q2", name="q2")
            hpi_t = pool.tile([P, 1], f32, name="hpi")

            nc.gpsimd.memset(hpi_t[:], PI / 2.0)
            # warm the ACT function tables while input DMAs run
            warm_t = pool.tile([P, 1], f32, name="warm")
            nc.scalar.activation(warm_t[:], hpi_t[:], AFT.Tanh)
            nc.scalar.activation(warm_t[:], hpi_t[:], AFT.Sin)

            # ---------------- input DMAs ----------------
            la_v = la_d[:].rearrange("(s l p) j -> p s l j", p=P, l=S_B)
            for s in range(NSUB):
                for ln in range(S_B):
                    nc.sync.dma_start(la_t[s][:, ln * J:(ln + 1) * J],
                                      la_v[:, s, ln])
            tq = (12 * M) // 4
            for c in range(4):
                tip_src = AP(tip_d, c * tq, [[0, P], [1, tq]])
                nc.sync.dma_start(tip_t[:, c * tq:(c + 1) * tq], tip_src)

            act = nc.scalar.activation

            for s in range(NSUB):
                lat = la_t[s][:]
                tht = th_t[s][:]
                # tanh with axis-deinterleave: iterate (a, e, lane)
                th_out = AP(tht.tensor, tht.offset,
                            [list(tht.ap[0]), [M, 3], [S_B, E], [1, S_B]])
                la_in = AP(lat.tensor, lat.offset,
                           [list(lat.ap[0]), [1, 3], [3, E], [J, S_B]])
                act(th_out, la_in, AFT.Tanh)
                # per-axis |theta| + sin/cos, all on ACT (abs/sin/tanh
                # share one ACT table set); z first (BC triples 1-2 need
                # only the z trig)
                for a in (2, 1, 0):
                    sl = slice(a * M, (a + 1) * M)
                    act(ab_t[s][:, sl], th_t[s][:, sl], AFT.Abs)
                    act(sin_t[s][:, sl], th_t[s][:, sl], AFT.Sin,
                        scale=sc_const)
                    act(cos_t[s][:, sl], ab_t[s][:, sl], AFT.Sin,
                        bias=hpi_t[:], scale=-sc_const)

            # ---------------- BC: locals into locT layout ----------------
            def trig4(tile, axis):
                t = tile[:]
                return AP(t.tensor, t.offset + axis * M,
                          [list(t.ap[0]), [0, 4], [1, M]])

            def row4(tile, k):
                t = tile[:]
                return AP(t.tensor, t.offset + k * 4 * M,
                          [list(t.ap[0]), [M, 4], [1, M]])

            def scr4(tile):
                t = tile[:]
                return AP(t.tensor, t.offset,
                          [list(t.ap[0]), [M, 4], [1, M]])

            def col_slice(ap, lo_e, n_e):
                dims = [list(d) for d in ap.ap]
                dims[-1] = [dims[-1][0], n_e]
                return AP(ap.tensor, ap.offset + lo_e, dims)

            for s in range(NSUB):
                sx, sy, sz = (trig4(sin_t[s], a) for a in range(3))
                cx, cy, cz = (trig4(cos_t[s], a) for a in range(3))
                T0, T1, T2 = (row4(tip_t, k) for k in range(3))
                L0, L1, L2 = (row4(loc_t[s], k) for k in range(3))
                r0v, r1v, q2v, tAv = (scr4(t) for t in
                                      (r0_t, r1_t, q2_t, tA_t))

                triples = [
                    (cz, T0, sz, T1, r0v, Alu.subtract, True),
                    (sz, T0, cz, T1, r1v, Alu.add, False),
                    (cy, r0v, sy, T2, L0, Alu.add, False),
                    (sy, r0v, cy, T2, q2v, Alu.subtract, False),
                    (cx, r1v, sx, q2v, L1, Alu.subtract, True),
                    (sx, r1v, cx, q2v, L2, Alu.add, False),
                ]
                for ti, (a, b, c, d, dst, op, ta_first) in enumerate(triples):
                    pf = BC_POOL_TRIPLES.get(ti, 0.0)
                    m_dve = M - int(M * pf)
                    for eng, lo_e, n_e in ((nc.vector, 0, m_dve),
                                           (nc.gpsimd, m_dve, M - m_dve)):
                        if n_e <= 0:
                            continue
                        tt = eng.tensor_tensor
                        aa, bb, cc, dd, dd_dst, tv = (
                            col_slice(x, lo_e, n_e)
                            for x in (a, b, c, d, dst, tAv))
                        tt(tv, aa, bb, Alu.mult)
                        tt(dd_dst, cc, dd, Alu.mult)
                        if ta_first:
                            tt(dd_dst, tv, dd_dst, op)
                        else:
                            tt(dd_dst, dd_dst, tv, op)

            # ---------------- lR replication ----------------
            rep_engs = []
            acc = 0.0
            for frac, eng in zip(REP_SPLIT, ("act", "pool", "dve")):
                lo_e = int(M * acc)
                acc += frac
                hi_e = M if acc >= 0.999 else int(M * acc)
                if hi_e > lo_e:
                    rep_engs.append((eng, lo_e, hi_e))

            # split each rep op at edge 127 (e' = 127*S_B): the FRONT range
            # covers everything tree levels 1-5 read, so the tree can start
            # as soon as the front lands; the BACK replicates during the
            # tree's early levels.  Removes the DVE stall where rep0 (one
            # big op per k) outlasted the BC work available to hide it.
            REP_CUT = 127 * S_B
            for s in range(NSUB):
                lt = loc_t[s][:]
                rt = lR_t[s][:]
                for k in range(3):
                    for eng, lo_e, hi_e in rep_engs:
                        cuts = [lo_e]
                        if lo_e < REP_CUT < hi_e:
                            cuts.append(REP_CUT)
                        cuts.append(hi_e)
                        for c0, c1 in zip(cuts[:-1], cuts[1:]):
                            n_e = c1 - c0
                            dst = AP(rt.tensor,
                                     rt.offset + c0 * 36 + k * 12,
                                     [list(rt.ap[0]), [36, n_e], [3, 4],
                                      [1, 3]])
                            src = AP(lt.tensor,
                                     lt.offset + k * 4 * M + c0,
                                     [list(lt.ap[0]), [1, n_e], [M, 4],
                                      [0, 3]])
                            if eng == "act":
                                nc.scalar.copy(dst, src)
                            elif eng == "pool":
                                nc.gpsimd.tensor_copy(dst, src)
                            else:
                                nc.vector.tensor_copy(dst, src)

            # ---------------- tree ----------------
            for s in range(NSUB):
                wt = w_t[s][:]
                rt = lR_t[s][:]
                tst = t_t[s][:]
                wpd, rpd, tpd = (list(x.ap[0]) for x in (wt, rt, tst))

                def wAP(off, dims):
                    return AP(wt.tensor, wt.offset + off, [list(wpd)] + dims)

                def rAP(off, dims):
                    return AP(rt.tensor, rt.offset + off, [list(rpd)] + dims)

                def tAP(off, dims):
                    return AP(tst.tensor, tst.offset + off,
                              [list(tpd)] + dims)

                def nd(n):
                    return (n - 1) * SB12

                # nodes 1,2 <- locals of edges 0,1 ((l,i) from lR's (k,l))
                for n in (1, 2):
                    dst = wAP(nd(n), [[12, S_B], [3, 4], [1, 3]])
                    src = rAP((n - 1) * S_B * 36,
                              [[36, S_B], [3, 4], [12, 3]])
                    nc.vector.tensor_copy(dst, src)

                levels = [(3, 7), (7, 15), (15, 31), (31, 63), (63, 127),
                          (127, 191), (191, 255), (255, 256)]

                def emit_group(eng, glo, ghi, lo, add_eng=None,
                               tr_eng=None):
                    """Full level-compute for child-node range [glo, ghi).

                    k=1/k=2 partials go to k-separated halves of the t
                    scratch so the k=2 muls (eng) never overlap the k=1
                    add (add_eng) — no cross-engine WAR stall.
                    """
                    gm = ghi - glo
                    if gm <= 0:
                        return
                    tt = eng.tensor_tensor
                    tta = (add_eng or eng).tensor_tensor
                    for k in range(3):
                        for side in (0, 1):
                            q = (gm + (1 - side)) // 2
                            if q <= 0:
                                continue
                            gplo = (glo + side - 1) // 2
                            in0 = wAP(nd(gplo) + k * 3,
                                      [[12, S_B * q], [0, 4], [1, 3]])
                            e0 = glo + side - 1
                            in1 = rAP(S_B * e0 * 36 + k * 12,
                                      [[72 * S_B, q], [36, S_B], [1, 12]])
                            if k == 0:
                                dst = wAP(nd(glo + side),
                                          [[24 * S_B, q], [12, S_B],
                                           [1, 12]])
                            else:
                                dst = tAP((k - 1) * 64 * SB12
                                          + (glo - lo + side) * SB12,
                                          [[24 * S_B, q], [12, S_B],
                                           [1, 12]])
                            tt(dst, in0, in1, Alu.mult)
                        if k > 0:
                            wa = wAP(nd(glo), [[1, SB12 * gm]])
                            ta = tAP((k - 1) * 64 * SB12
                                     + (glo - lo) * SB12, [[1, SB12 * gm]])
                            tta(wa, wa, ta, Alu.add)
                    # translation add: w[child].t += w[parent].t
                    # (a trailing side-chain: next level's muls read only
                    # rotation rows, so tr can run on another engine)
                    ttr = (tr_eng or eng).tensor_tensor
                    for side in (0, 1):
                        q = (gm + (1 - side)) // 2
                        if q <= 0:
                            continue
                        gplo = (glo + side - 1) // 2
                        wtr = wAP(nd(glo + side) + 9,
                                  [[24 * S_B, q], [12, S_B], [1, 3]])
                        ptr = wAP(nd(gplo) + 9, [[12, S_B * q], [1, 3]])
                        ttr(wtr, wtr, ptr, Alu.add)

                for (lo, hi) in levels:
                    m = hi - lo
                    if LAST_LEVEL_POOL and lo >= 191:
                        # whole tail level on POOL: frees DVE at the
                        # iteration tail; latency hides under the next
                        # iteration's (DVE-only) BC phase.
                        emit_group(nc.gpsimd, lo, hi, lo)
                        continue
                    if m >= TREE_POOL_MIN_M and TREE_POOL_FRAC > 0:
                        mid = hi - int(m * TREE_POOL_FRAC)
                        mid += (hi - mid) % 2
                    else:
                        mid = hi
                    a_eng = (nc.gpsimd if m >= TREE_ADD_POOL_MIN_M
                             else None)
                    t_eng = nc.gpsimd if TR_POOL else None
                    emit_group(nc.vector, lo, mid, lo, add_eng=a_eng,
                               tr_eng=t_eng)
                    emit_group(nc.gpsimd, mid, hi, lo)

            # ---------------- output DMAs ----------------
            # staged: nodes 1..126 (cols [0, 126*SB12)) are final after level
            # (63,127); tile deps let those DMAs start while the tree tail
            # still runs.  Remainder flushed after the full tree.
            ov = out_d[:]
            w_cols = 255 * SB12
            c1 = 126 * SB12    # nodes 1..126: final after level (63,127)
            c2 = 190 * SB12    # nodes 127..190: final after (127,191)
            for s in range(NSUB):
                for c0, cn in ((0, c1), (c1, c2 - c1), (c2, w_cols - c2)):
                    dst = AP(ov.tensor, ov.offset + s * w_cols + c0,
                             [list(ov.ap[0]), [1, cn]])
                    nc.sync.dma_start(dst, w_t[s][:, c0:c0 + cn])

    nc.compile()
    return nc


# --------------------------------------------------------------------------- #
# cached PJRT runner (axon path) — compile once, execute per call
# --------------------------------------------------------------------------- #
def _get_runner(sc_const, loop_n=1):
    key = ("runner", round(sc_const, 6), loop_n)
    if key in _state:
        return _state[key]

    import jax
    from jax.sharding import Mesh, PartitionSpec, NamedSharding
    from jax.experimental.shard_map import shard_map
    import concourse.mybir as mybir
    from concourse import bass2jax

    nc = _build_nc(sc_const, loop_n)
    bass2jax.install_neuronx_cc_hook()

    part_name = (nc.partition_id_tensor.name
                 if nc.partition_id_tensor is not None else None)
    in_names, out_names, out_avals = [], [], []
    for alloc in nc.m.functions[0].allocations:
        if not isinstance(alloc, mybir.MemoryLocationSet):
            continue
        name = alloc.memorylocations[0].name
        if alloc.kind == "ExternalInput":
            if name != part_name:
                in_names.append(name)
        elif alloc.kind == "ExternalOutput":
            out_names.append(name)
            out_avals.append(jax.core.ShapedArray(
                tuple(alloc.tensor_shape), mybir.dt.np(alloc.dtype)))
    n_params = len(in_names)
    all_in = in_names + out_names
    if part_name is not None:
        all_in = all_in + [part_name]

    def _body(*args):
        operands = list(args)
        if part_name is not None:
            operands.append(bass2jax.partition_id_tensor())
        outs = bass2jax._bass_exec_p.bind(
            *operands,
            out_avals=tuple(out_avals),
            in_names=tuple(all_in),
            out_names=tuple(out_names),
            lowering_input_output_aliases=(),
            sim_require_finite=True,
            sim_require_nnan=True,
            nc=nc,
        )
        return tuple(outs)

    devices = jax.devices()[:NCORE]
    mesh = Mesh(np.asarray(devices), ("core",))
    nin = n_params + len(out_names)
    sharded = jax.jit(
        shard_map(_body, mesh=mesh,
                  in_specs=(PartitionSpec("core"),) * nin,
                  out_specs=(PartitionSpec("core"),) * len(out_names),
                  check_rep=False),
        donate_argnums=tuple(range(n_params, nin)),
        keep_unused=True,
    )
    shard0 = NamedSharding(mesh, PartitionSpec("core"))

    def _make_zeros():
        return jax.jit(
            lambda: jax.numpy.zeros((NCORE * P, OUTC),
                        np.float16 if F16 else np.float32),
            out_shardings=shard0)()

    runner = (sharded, in_names, _make_zeros)
    _state[key] = runner
    return runner


def _prep_tip(tip_to_base):
    # tipT[k, l, e'] with e' = S_B*e + lane (lane-duplicated), compute dtype
    cdt = np.float16 if F16 else np.float32
    tip_rows = tip_to_base[:, :3, :].astype(cdt)             # (E, 3, 4)
    tipT = np.repeat(tip_rows.transpose(1, 2, 0), S_B, axis=-1)  # (3,4,M)
    return np.ascontiguousarray(tipT.reshape(1, 12 * M))


def _run_device(log_angles, tip_to_base, sc_const):
    sharded, in_names, make_zeros = _get_runner(sc_const)
    feed = {
        "la": np.ascontiguousarray(log_angles, dtype=np.float32),
        "tipT": np.broadcast_to(_prep_tip(tip_to_base),
                                (NCORE, 12 * M)).copy(),
    }
    args = [feed[name] for name in in_names]
    out = np.asarray(sharded(*args, make_zeros())[0])
    # out: (NCORE*P, OUTC) fp16, device layout -> (B, N, 4, 4) fp32
    v = out.reshape(NCORE, P, NSUB, 255, S_B, 4, 3)  # (c, p, s, n, lane, l, i)
    rot = v.transpose(0, 2, 4, 1, 3, 6, 5)           # (c, s, lane, p, n, i, l)
    res = np.zeros((B, N, 4, 4), np.float32)
    res[:, 1:, :3, :] = rot.reshape(B, 255, 3, 4).astype(np.float32)
    res[:, 0, 0, 0] = 1.0
    res[:, 0, 1, 1] = 1.0
    res[:, 0, 2, 2] = 1.0
    res[:, :, 3, 3] = 1.0
    return res


# --------------------------------------------------------------------------- #
# public entry point
# --------------------------------------------------------------------------- #
def kernel(log_angles, tip_to_base, rot_axes, rot_constraints):
    log_angles = np.asarray(log_angles)
    tip_to_base = np.asarray(tip_to_base)
    rot_axes = np.asarray(rot_axes)
    rot_constraints = np.asarray(rot_constraints)

    expected_shapes = (log_angles.shape == (B, J)
                       and tip_to_base.shape == (E, 4, 4)
                       and rot_axes.shape == (J, 3)
                       and rot_constraints.shape == (J, 2))
    eye_tiled = np.tile(np.eye(3, dtype=np.float32), (E, 1)) \
        if expected_shapes else None
    euler = expected_shapes and np.allclose(rot_axes, eye_tiled, atol=1e-6)
    if not euler:
        return _np_fallback(log_angles, tip_to_base, rot_axes, rot_constraints)

    sc = rot_constraints[:, 0].astype(np.float32)
    of = rot_constraints[:, 1].astype(np.float32)
    const_ok = (np.all(sc == sc[0]) and np.all(of == 0.0)
                and float(sc[0]) > 1e-3
                and abs(float(sc[0])) <= PI + 1e-4)
    if not const_ok:
        return _np_fallback(log_angles, tip_to_base, rot_axes,
                            rot_constraints)

    return _run_device(log_angles, tip_to_base, float(sc[0]))
